# revision 7
# baseline (speedup 1.0000x reference)
"""FP4 (E2M1) fake-quant Linear: y = fq(x) @ fq(W) + b on 8 NeuronCores.

Strategy: data-parallel over x rows (16384/8 = 2048 rows/core); each core
quantizes its own x shard and a 512-column slice of W, AllGathers the
quantized (fp8-encoded FP4 level) weights, and computes its y rows with
fp8 DoubleRow matmuls. The FP4 quantize-dequantize is done exactly (same
decision boundaries as the reference's searchsorted round-to-nearest) via
magic-number rounding, with the matmul carried out on the integer-like
level values and the combined scale applied at PSUM eviction.

Engine placement: ACT does abs/sign, DVE the magic-add rounding
(2x-mode tensor_scalars), GpSimd the chain adds/sign-multiply, PE the
bf16 transposes + DoubleRow matmuls, any-engine the PSUM->SBUF copies.
"""

import numpy as np

import concourse.bass as bass
import concourse.bass_isa as bass_isa
import concourse.mybir as mybir
import concourse.tile as tile
from concourse import bacc
from concourse.bass_utils import run_bass_kernel_spmd
from concourse.masks import make_identity

F32 = mybir.dt.float32
FP8 = mybir.dt.float8e4
BF16 = mybir.dt.bfloat16
AF = mybir.ActivationFunctionType
ALU = mybir.AluOpType
AX = mybir.AxisListType

P = 128
N_CORES = 8

# magic rounding constants (f32): adding C with ulp(C)=s rounds to spacing s
C1 = float(np.float32(1.5 * 2**23 * 0.5))   # spacing 0.5
C2 = float(np.float32(1.5 * 2**23 * 1.0))   # spacing 1.0


def _quant_chain(nc, pools, src, out, r_ap):
    """FP4-level quantize: out = sign(src) * fp4_round(|src| * r_ap).

    out dtype may be bf16 or fp8 (levels exact in both). Matches the
    reference searchsorted semantics except on measure-zero ties.
    """
    shape = [src.shape[0], src.free_size()]
    a = pools["qa"].tile(shape, F32, tag="qa")
    sgn = pools["qs"].tile(shape, F32, tag="qs")
    u1 = pools["qu1"].tile(shape, F32, tag="qu1")
    u2 = pools["qu2"].tile(shape, F32, tag="qu2")
    v3 = pools["qv3"].tile(shape, F32, tag="qv3")
    nc.scalar.activation(a[:], src, AF.Abs, scale=r_ap)
    nc.scalar.activation(sgn[:], src, AF.Sign)
    nc.vector.tensor_scalar(u1[:], a[:], 2.0, C1, ALU.min, ALU.add)
    nc.vector.tensor_scalar(u2[:], a[:], 4.0, C2, ALU.min, ALU.add)
    # v2 = r1(clip(a,2,4)) - 2 - C1   (in-place over u2)
    nc.vector.tensor_scalar(u2[:], u2[:], C2 + 2.0, C1 + C2 + 2.0, ALU.max, ALU.subtract)
    nc.vector.tensor_scalar(v3[:], a[:], 5.0, 2.0, ALU.is_gt, ALU.mult)
    nc.vector.tensor_tensor(u1[:], u1[:], u2[:], ALU.add)   # t  (in-place)
    nc.gpsimd.tensor_tensor(u1[:], u1[:], v3[:], ALU.add)   # t2 (in-place)
    nc.gpsimd.tensor_tensor(out, u1[:], sgn[:], ALU.mult)   # bf16/fp8 out


def build_nc(m_sh, k, n, with_bias, n_cores=N_CORES):
    ko = k // P          # 128-row K blocks
    k2 = ko // 2         # DoubleRow K block pairs
    mt = m_sh // P       # m-chunks
    n_sh = n // n_cores  # gathered slice width per core
    nh = n // 2          # N half width
    nq = nh // 512       # 512-wide psum chunks per half
    assert nh % 512 == 0 and nh % n_sh == 0
    cores = list(range(n_cores))

    nc = bacc.Bacc(None, target_bir_lowering=False, debug=False)

    x_in = nc.declare_dram_parameter("x", [m_sh, k], F32, isOutput=False)
    w_in = nc.declare_dram_parameter("w", [k, n_sh], F32, isOutput=False)
    b_in = nc.declare_dram_parameter("b", [n], F32, isOutput=False)
    y_out = nc.declare_dram_parameter("y", [m_sh, n], F32, isOutput=True)

    ag_in = nc.dram_tensor("ag_in", [ko, P, n_sh], FP8)
    ag_out = nc.dram_tensor("ag_out", [n_cores, ko, P, n_sh], FP8, addr_space="Shared")
    ar_in = nc.dram_tensor("ar_in", [1, 2], F32)
    ar_out = nc.dram_tensor("ar_out", [1, 2], F32, addr_space="Shared")

    x_t = x_in.rearrange("(mt p) k -> mt p k", p=P)

    with tile.TileContext(nc) as tc:
        with (
            tc.tile_pool(name="const", bufs=1) as const,
            tc.tile_pool(name="stat", bufs=1) as stat,
            tc.tile_pool(name="qa", bufs=2) as qa_p,
            tc.tile_pool(name="qs", bufs=2) as qs_p,
            tc.tile_pool(name="qu1", bufs=2) as qu1_p,
            tc.tile_pool(name="qu2", bufs=2) as qu2_p,
            tc.tile_pool(name="qv3", bufs=2) as qv3_p,
            tc.tile_pool(name="lwq", bufs=2) as lwq_p,
        ):
            pools = {"qa": qa_p, "qs": qs_p, "qu1": qu1_p, "qu2": qu2_p, "qv3": qv3_p}

            ident = const.tile([P, P], BF16)
            make_identity(nc, ident)
            if with_bias:
                bias_bc = const.tile([P, n], F32)
                nc.sync.dma_start(bias_bc[:], b_in[None, :].to_broadcast((P, n)))
            else:
                # still consume the input so the NEFF binds it
                bias_1 = const.tile([1, n], F32)
                nc.sync.dma_start(bias_1[:], b_in[None, :])

            # ---- Phase A: local absmax + cross-core AllReduce(max) ----
            with tc.tile_pool(name="wk", bufs=1) as wk_p, tc.tile_pool(name="xa", bufs=3) as xa_p:
                w_keep = wk_p.tile([P, ko, n_sh], F32)
                w_src = w_in.rearrange("(ko ki) n -> ki ko n", ki=P)
                for g in range(0, ko, 8):
                    nc.sync.dma_start(w_keep[:, g:g + 8], w_src[:, g:g + 8])
                wmax_c = stat.tile([P, ko], F32)
                nc.vector.tensor_reduce(wmax_c[:], w_keep[:], AX.X, ALU.max, apply_absolute_value=True)
                both = stat.tile([P, 2], F32)
                nc.vector.tensor_reduce(both[:, 1:2], wmax_c[:], AX.X, ALU.max)

                xmax_c = stat.tile([P, mt], F32)
                for mi in range(mt):
                    xa = xa_p.tile([P, k], F32, tag="xa")
                    nc.sync.dma_start(xa[:], x_t[mi])
                    nc.vector.tensor_reduce(xmax_c[:, mi:mi + 1], xa[:], AX.X, ALU.max, apply_absolute_value=True)
                nc.vector.tensor_reduce(both[:, 0:1], xmax_c[:], AX.X, ALU.max)

                both_r = stat.tile([P, 2], F32)
                nc.gpsimd.partition_all_reduce(both_r[:], both[:], P, bass_isa.ReduceOp.max)
                nc.sync.dma_start(ar_in[:], both_r[0:1, :])
                nc.gpsimd.collective_compute(
                    "AllReduce", ALU.max, replica_groups=[cores],
                    ins=[ar_in[:]], outs=[ar_out[:]],
                )
                # scales: s = max(amax/6, 1e-12); r = 1/s (Newton-refined recip)
                sc = stat.tile([1, 2], F32)
                nc.sync.dma_start(sc[:], ar_out[:])
                s_t = stat.tile([1, 2], F32)
                nc.vector.tensor_scalar(s_t[:], sc[:], 1.0 / 6.0, 1e-12, ALU.mult, ALU.max)
                r0 = stat.tile([1, 2], F32)
                nc.vector.reciprocal(r0[:], s_t[:])
                t1 = stat.tile([1, 2], F32)
                nc.vector.tensor_tensor(t1[:], s_t[:], r0[:], ALU.mult)
                nc.vector.tensor_scalar(t1[:], t1[:], 2.0, -1.0, ALU.subtract, ALU.mult)
                nc.vector.tensor_tensor(r0[:], r0[:], t1[:], ALU.mult)
                sxw = stat.tile([1, 1], F32)
                nc.vector.tensor_tensor(sxw[:], s_t[:, 0:1], s_t[:, 1:2], ALU.mult)
                rb = stat.tile([P, 2], F32)
                nc.gpsimd.partition_broadcast(rb[:], r0[:])
                sxwb = stat.tile([P, 1], F32)
                nc.gpsimd.partition_broadcast(sxwb[:], sxw[:])

                # ---- Phase B: quantize w slice, AllGather ----
                for ki in range(ko):
                    lwq = lwq_p.tile([P, n_sh], FP8, tag="lwq")
                    _quant_chain(nc, pools, w_keep[:, ki], lwq[:], rb[:, 1:2])
                    nc.sync.dma_start(ag_in[ki], lwq[:])
                nc.gpsimd.collective_compute(
                    "AllGather", ALU.bypass, replica_groups=[cores],
                    ins=[ag_in[:]], outs=[ag_out[:]],
                )

            # ---- Phase C+D: x quantize+transpose one chunk ahead of matmul ----
            with (
                tc.tile_pool(name="lxT", bufs=1) as lxt_p,
                tc.tile_pool(name="lw", bufs=1) as lw_p,
                tc.tile_pool(name="xq", bufs=3) as xq_p,
                tc.tile_pool(name="lx", bufs=2) as lx_p,
                tc.tile_pool(name="tp_psum", bufs=3, space="PSUM") as tp_p,
                tc.tile_pool(name="mm_psum", bufs=5, space="PSUM") as mm_p,
                tc.tile_pool(name="ystg", bufs=2) as ys_p,
            ):
                lxT = lxt_p.tile([P, ko, m_sh], FP8)

                def quant_transpose(m):
                    for q in range(k // 1024):
                        xq = xq_p.tile([P, 1024], F32, tag="xq")
                        nc.sync.dma_start(xq[:], x_t[m, :, q * 1024:(q + 1) * 1024])
                        lx = lx_p.tile([P, 1024], BF16, tag="lx")
                        _quant_chain(nc, pools, xq[:], lx[:], rb[:, 0:1])
                        for g in range(2):
                            pt = tp_p.tile([P, 512], BF16, tag="tp")
                            for j in range(4):
                                nc.tensor.transpose(
                                    pt[:, j * P:(j + 1) * P],
                                    lx[:, (g * 4 + j) * P:(g * 4 + j + 1) * P],
                                    ident[:],
                                )
                            ko0 = q * 8 + g * 4
                            nc.any.tensor_copy(
                                lxT[:, ko0:ko0 + 4, m * P:(m + 1) * P], pt[:],
                            )

                def load_lw_half(h):
                    lw_h = lw_p.tile([P, ko, nh], FP8, tag="lwh")
                    spc = n_sh  # slice width contributed by each core
                    for c in range(nh // spc):
                        src = ag_out[h * (nh // spc) + c]  # [ko, P, n_sh]
                        nc.sync.dma_start(
                            lw_h[:, :, c * spc:(c + 1) * spc],
                            src.rearrange("ko ki n -> ki ko n"),
                        )
                    return lw_h

                def mm_body(m, h, lw_h):
                    psums = [mm_p.tile([P, 512], F32, tag="mm", name=f"mm_{m}_{h}_{i}") for i in range(nq)]
                    for kk in range(k2):
                        lhsT = lxT[:, 2 * kk:2 * kk + 2, m * P:(m + 1) * P]
                        for q in range(nq):
                            nc.tensor.matmul(
                                psums[q][:], lhsT,
                                lw_h[:, 2 * kk:2 * kk + 2, q * 512:(q + 1) * 512],
                                start=(kk == 0), stop=(kk == k2 - 1),
                                perf_mode=mybir.MatmulPerfMode.DoubleRow,
                            )
                    for q in range(nq):
                        ys = ys_p.tile([P, 512], F32, tag="ys")
                        nc.scalar.activation(ys[:], psums[q][:], AF.Copy, scale=sxwb[:])
                        col = h * nh + q * 512
                        if with_bias:
                            nc.gpsimd.tensor_tensor(ys[:], ys[:], bias_bc[:, col:col + 512], ALU.add)
                        nc.sync.dma_start(y_out[m * P:(m + 1) * P, col:col + 512], ys[:])

                lw_h0 = load_lw_half(0)
                quant_transpose(0)
                for m in range(mt):
                    if m + 1 < mt:
                        quant_transpose(m + 1)
                    mm_body(m, 0, lw_h0)
                lw_h1 = load_lw_half(1)
                for m in range(mt):
                    mm_body(m, 1, lw_h1)

    nc.compile()
    return nc


_NC_CACHE = {}


def _get_nc(m_sh, k, n, with_bias):
    key = (m_sh, k, n, with_bias)
    if key not in _NC_CACHE:
        _NC_CACHE[key] = build_nc(m_sh, k, n, with_bias)
    return _NC_CACHE[key]


def kernel(x, weight, bias, _trace=False, _tmpdir=None):
    x = np.ascontiguousarray(np.asarray(x, dtype=np.float32))
    weight = np.ascontiguousarray(np.asarray(weight, dtype=np.float32))
    bias = np.ascontiguousarray(np.asarray(bias, dtype=np.float32))
    m, k = x.shape
    _, n = weight.shape
    m_sh = m // N_CORES
    n_sh = n // N_CORES
    with_bias = bool(np.any(bias != 0.0))
    nc = _get_nc(m_sh, k, n, with_bias)
    in_maps = [
        {
            "x": x[i * m_sh:(i + 1) * m_sh],
            "w": np.ascontiguousarray(weight[:, i * n_sh:(i + 1) * n_sh]),
            "b": bias,
        }
        for i in range(N_CORES)
    ]
    res = run_bass_kernel_spmd(nc, in_maps, core_ids=list(range(N_CORES)), trace=_trace, tmpdir=_tmpdir)
    out = np.concatenate([res.results[i]["y"] for i in range(N_CORES)], axis=0)
    if _trace:
        return out, res
    return out


# revision 11
# speedup vs baseline: 1.1257x; 1.1257x over previous
"""FP4 (E2M1) fake-quant Linear: y = fq(x) @ fq(W) + b on 8 NeuronCores.

Strategy: data-parallel over x rows (16384/8 = 2048 rows/core); each core
quantizes its own x shard and a 512-column slice of W, AllGathers the
quantized (fp8-encoded FP4 level) weights, and computes its y rows with
fp8 DoubleRow matmuls. The FP4 quantize-dequantize is done exactly (same
decision boundaries as the reference's searchsorted round-to-nearest) via
magic-number rounding, with the matmul carried out on the integer-like
level values and the combined scale applied at PSUM eviction.

Engine placement: ACT does abs/sign, DVE the magic-add rounding
(2x-mode tensor_scalars), GpSimd the chain adds/sign-multiply, PE the
bf16 transposes + DoubleRow matmuls, any-engine the PSUM->SBUF copies.
"""

import numpy as np

import concourse.bass as bass
import concourse.bass_isa as bass_isa
import concourse.mybir as mybir
import concourse.tile as tile
from concourse import bacc
from concourse.bass_utils import run_bass_kernel_spmd
from concourse.masks import make_identity

F32 = mybir.dt.float32
FP8 = mybir.dt.float8e4
BF16 = mybir.dt.bfloat16
AF = mybir.ActivationFunctionType
ALU = mybir.AluOpType
AX = mybir.AxisListType

P = 128
N_CORES = 8

# magic rounding constants (f32): adding C with ulp(C)=s rounds to spacing s
C1 = float(np.float32(1.5 * 2**23 * 0.5))   # spacing 0.5
C2 = float(np.float32(1.5 * 2**23 * 1.0))   # spacing 1.0


def _quant_chain(nc, pools, src, out, r_ap, gps=True):
    """FP4-level quantize: out = sign(src) * fp4_round(|src| * r_ap).

    out dtype may be bf16 or fp8 (levels exact in both). Matches the
    reference searchsorted semantics except on measure-zero ties.
    """
    shape = [src.shape[0], src.free_size()]
    a = pools["qa"].tile(shape, F32, tag="qa")
    sgn = pools["qs"].tile(shape, F32, tag="qs")
    u1 = pools["qu1"].tile(shape, F32, tag="qu1")
    u2 = pools["qu2"].tile(shape, F32, tag="qu2")
    v3 = pools["qv3"].tile(shape, F32, tag="qv3")
    nc.scalar.activation(a[:], src, AF.Abs, scale=r_ap)
    nc.scalar.activation(sgn[:], src, AF.Sign)
    nc.vector.tensor_scalar(u1[:], a[:], 2.0, C1, ALU.min, ALU.add)
    nc.vector.tensor_scalar(u2[:], a[:], 4.0, C2, ALU.min, ALU.add)
    # v2 = r1(clip(a,2,4)) - 2 - C1   (in-place over u2)
    nc.vector.tensor_scalar(u2[:], u2[:], C2 + 2.0, C1 + C2 + 2.0, ALU.max, ALU.subtract)
    nc.vector.tensor_scalar(v3[:], a[:], 5.0, 2.0, ALU.is_gt, ALU.mult)
    nc.vector.tensor_tensor(u1[:], u1[:], u2[:], ALU.add)   # t  (in-place)
    nc.vector.tensor_tensor(u1[:], u1[:], v3[:], ALU.add)   # t2 (in-place)
    nc.vector.tensor_tensor(out, u1[:], sgn[:], ALU.mult)   # bf16/fp8 out


def build_nc(m_sh, k, n, with_bias, n_cores=N_CORES):
    ko = k // P          # 128-row K blocks
    k2 = ko // 2         # DoubleRow K block pairs
    mt = m_sh // P       # m-chunks
    n_sh = n // n_cores  # gathered slice width per core
    nh = n // 2          # N half width
    nq = nh // 512       # 512-wide psum chunks per half
    assert nh % 512 == 0 and nh % n_sh == 0
    cores = list(range(n_cores))

    nc = bacc.Bacc(None, target_bir_lowering=False, debug=False)

    x_in = nc.declare_dram_parameter("x", [m_sh, k], F32, isOutput=False)
    w_in = nc.declare_dram_parameter("w", [k, n_sh], F32, isOutput=False)
    b_in = nc.declare_dram_parameter("b", [n], F32, isOutput=False)
    y_out = nc.declare_dram_parameter("y", [m_sh, n], F32, isOutput=True)

    ag_in = nc.dram_tensor("ag_in", [ko, P, n_sh], FP8)
    ag_out = nc.dram_tensor("ag_out", [n_cores, ko, P, n_sh], FP8, addr_space="Shared")
    arw_in = nc.dram_tensor("arw_in", [1, 1], F32)
    arw_out = nc.dram_tensor("arw_out", [1, 1], F32, addr_space="Shared")
    arx_in = nc.dram_tensor("arx_in", [1, 1], F32)
    arx_out = nc.dram_tensor("arx_out", [1, 1], F32, addr_space="Shared")

    x_t = x_in.rearrange("(mt p) k -> mt p k", p=P)

    with tile.TileContext(nc) as tc:
        with (
            tc.tile_pool(name="const", bufs=1) as const,
            tc.tile_pool(name="stat", bufs=1) as stat,
            tc.tile_pool(name="qa", bufs=2) as qa_p,
            tc.tile_pool(name="qs", bufs=2) as qs_p,
            tc.tile_pool(name="qu1", bufs=2) as qu1_p,
            tc.tile_pool(name="qu2", bufs=2) as qu2_p,
            tc.tile_pool(name="qv3", bufs=2) as qv3_p,
            tc.tile_pool(name="lwq", bufs=2) as lwq_p,
        ):
            pools = {"qa": qa_p, "qs": qs_p, "qu1": qu1_p, "qu2": qu2_p, "qv3": qv3_p}

            ident = const.tile([P, P], F32)
            make_identity(nc, ident)
            if with_bias:
                bias_bc = const.tile([P, n], F32)
                nc.sync.dma_start(bias_bc[:], b_in[None, :].to_broadcast((P, n)))
            else:
                # still consume the input so the NEFF binds it
                bias_1 = const.tile([1, n], F32)
                nc.sync.dma_start(bias_1[:], b_in[None, :])

            # ---- Phase A/B: absmax + scales + w quantize + AllGather ----
            # w side first: its AllReduce+quantize+AllGather overlap the x scan
            with tc.tile_pool(name="wk", bufs=1) as wk_p, tc.tile_pool(name="xa", bufs=3) as xa_p:
                w_keep = wk_p.tile([P, ko, n_sh], F32)
                w_src = w_in.rearrange("(ko ki) n -> ki ko n", ki=P)
                for g in range(0, ko, 8):
                    nc.sync.dma_start(w_keep[:, g:g + 8], w_src[:, g:g + 8])
                wmax_c = stat.tile([P, ko], F32)
                nc.vector.tensor_reduce(wmax_c[:], w_keep[:], AX.X, ALU.max, apply_absolute_value=True)
                wmax = stat.tile([P, 1], F32)
                nc.vector.tensor_reduce(wmax[:], wmax_c[:], AX.X, ALU.max)
                wmax_r = stat.tile([P, 1], F32)
                nc.gpsimd.partition_all_reduce(wmax_r[:], wmax[:], P, bass_isa.ReduceOp.max)
                nc.sync.dma_start(arw_in[:], wmax_r[0:1, :])
                nc.gpsimd.collective_compute(
                    "AllReduce", ALU.max, replica_groups=[cores],
                    ins=[arw_in[:]], outs=[arw_out[:]],
                )

                def scales(ar_out_t, tag):
                    sc = stat.tile([1, 1], F32, name=f"sc_{tag}")
                    nc.sync.dma_start(sc[:], ar_out_t[:])
                    s_t = stat.tile([1, 1], F32, name=f"s_{tag}")
                    nc.vector.tensor_scalar(s_t[:], sc[:], 1.0 / 6.0, 1e-12, ALU.mult, ALU.max)
                    r0 = stat.tile([1, 1], F32, name=f"r0_{tag}")
                    nc.vector.reciprocal(r0[:], s_t[:])
                    t1 = stat.tile([1, 1], F32, name=f"t1_{tag}")
                    nc.vector.tensor_tensor(t1[:], s_t[:], r0[:], ALU.mult)
                    nc.vector.tensor_scalar(t1[:], t1[:], 2.0, -1.0, ALU.subtract, ALU.mult)
                    nc.vector.tensor_tensor(r0[:], r0[:], t1[:], ALU.mult)
                    rb = stat.tile([P, 1], F32, name=f"rb_{tag}")
                    nc.gpsimd.partition_broadcast(rb[:], r0[:])
                    return s_t, rb

                s_w, rbw = scales(arw_out, "w")

                # w quantize (DVE/ACT only: GpSimd queue stays clear for x) + AllGather
                for ki in range(ko):
                    lwq = lwq_p.tile([P, n_sh], FP8, tag="lwq")
                    _quant_chain(nc, pools, w_keep[:, ki], lwq[:], rbw[:], gps=False)
                    nc.sync.dma_start(ag_in[ki], lwq[:])
                nc.gpsimd.collective_compute(
                    "AllGather", ALU.bypass, replica_groups=[cores],
                    ins=[ag_in[:]], outs=[ag_out[:]],
                )

                # x absmax scan (33.5MB) runs concurrently with all of the above
                xmax_c = stat.tile([P, mt], F32)
                for mi in range(mt):
                    xa = xa_p.tile([P, k], F32, tag="xa")
                    nc.sync.dma_start(xa[:], x_t[mi])
                    nc.vector.tensor_reduce(xmax_c[:, mi:mi + 1], xa[:], AX.X, ALU.max, apply_absolute_value=True)
                xmax = stat.tile([P, 1], F32)
                nc.vector.tensor_reduce(xmax[:], xmax_c[:], AX.X, ALU.max)
                xmax_r = stat.tile([P, 1], F32)
                nc.gpsimd.partition_all_reduce(xmax_r[:], xmax[:], P, bass_isa.ReduceOp.max)
                # serialize AR_x behind the AllGather (HW can't run two
                # collectives concurrently): add a zero contribution read
                # from the AllGather output
                agd = stat.tile([1, 1], FP8)
                nc.sync.dma_start(agd[:], ag_out[0, 0, 0:1, 0:1])
                agz = stat.tile([1, 1], F32)
                nc.vector.tensor_scalar(agz[:], agd[:], 0.0, None, ALU.mult)
                nc.vector.tensor_tensor(xmax_r[0:1, :], xmax_r[0:1, :], agz[:], ALU.add)
                nc.sync.dma_start(arx_in[:], xmax_r[0:1, :])
                nc.gpsimd.collective_compute(
                    "AllReduce", ALU.max, replica_groups=[cores],
                    ins=[arx_in[:]], outs=[arx_out[:]],
                )
                s_x, rbx = scales(arx_out, "x")

                sxw = stat.tile([1, 1], F32)
                nc.vector.tensor_tensor(sxw[:], s_x[:], s_w[:], ALU.mult)
                sxwb = stat.tile([P, 1], F32)
                nc.gpsimd.partition_broadcast(sxwb[:], sxw[:])

            # ---- Phase C+D: x quantize+transpose one chunk ahead of matmul ----
            with (
                tc.tile_pool(name="lxT", bufs=1) as lxt_p,
                tc.tile_pool(name="lw", bufs=1) as lw_p,
                tc.tile_pool(name="xq", bufs=3) as xq_p,
                tc.tile_pool(name="tp_psum", bufs=3, space="PSUM") as tp_p,
                tc.tile_pool(name="mm_psum", bufs=5, space="PSUM") as mm_p,
                tc.tile_pool(name="ystg", bufs=2) as ys_p,
            ):
                lxT = lxt_p.tile([P, mt, ko, P], FP8)

                def quant_transpose(m):
                    for q in range(k // 1024):
                        xq = xq_p.tile([P, 1024], F32, tag="xq")
                        nc.sync.dma_start(xq[:], x_t[m, :, q * 1024:(q + 1) * 1024])
                        for g in range(2):
                            pt = tp_p.tile([P, 512], F32, tag="tp")
                            for j in range(4):
                                nc.tensor.transpose(
                                    pt[:, j * P:(j + 1) * P],
                                    xq[:, (g * 4 + j) * P:(g * 4 + j + 1) * P],
                                    ident[:],
                                )
                            ko0 = q * 8 + g * 4
                            _quant_chain(nc, pools, pt[:],
                                         lxT[:, m, ko0:ko0 + 4, :], rbx[:])

                def load_lw_half(h):
                    lw_h = lw_p.tile([P, ko, nh], FP8, tag="lwh")
                    spc = n_sh  # slice width contributed by each core
                    for c in range(nh // spc):
                        src = ag_out[h * (nh // spc) + c]  # [ko, P, n_sh]
                        nc.sync.dma_start(
                            lw_h[:, :, c * spc:(c + 1) * spc],
                            src.rearrange("ko ki n -> ki ko n"),
                        )
                    return lw_h

                def mm_body(m, h, lw_h):
                    psums = [mm_p.tile([P, 512], F32, tag="mm", name=f"mm_{m}_{h}_{i}") for i in range(nq)]
                    for kk in range(k2):
                        lhsT = lxT[:, m, 2 * kk:2 * kk + 2, :]
                        for q in range(nq):
                            nc.tensor.matmul(
                                psums[q][:], lhsT,
                                lw_h[:, 2 * kk:2 * kk + 2, q * 512:(q + 1) * 512],
                                start=(kk == 0), stop=(kk == k2 - 1),
                                perf_mode=mybir.MatmulPerfMode.DoubleRow,
                            )
                    for q in range(nq):
                        ys = ys_p.tile([P, 512], F32, tag="ys")
                        nc.scalar.activation(ys[:], psums[q][:], AF.Copy, scale=sxwb[:])
                        col = h * nh + q * 512
                        if with_bias:
                            nc.gpsimd.tensor_tensor(ys[:], ys[:], bias_bc[:, col:col + 512], ALU.add)
                        nc.sync.dma_start(y_out[m * P:(m + 1) * P, col:col + 512], ys[:])

                lw_h0 = load_lw_half(0)
                quant_transpose(0)
                for m in range(mt):
                    if m + 1 < mt:
                        quant_transpose(m + 1)
                    mm_body(m, 0, lw_h0)
                lw_h1 = load_lw_half(1)
                for m in range(mt):
                    mm_body(m, 1, lw_h1)

    nc.compile()
    return nc


_NC_CACHE = {}


def _get_nc(m_sh, k, n, with_bias):
    key = (m_sh, k, n, with_bias)
    if key not in _NC_CACHE:
        _NC_CACHE[key] = build_nc(m_sh, k, n, with_bias)
    return _NC_CACHE[key]


def kernel(x, weight, bias, _trace=False, _tmpdir=None):
    x = np.ascontiguousarray(np.asarray(x, dtype=np.float32))
    weight = np.ascontiguousarray(np.asarray(weight, dtype=np.float32))
    bias = np.ascontiguousarray(np.asarray(bias, dtype=np.float32))
    m, k = x.shape
    _, n = weight.shape
    m_sh = m // N_CORES
    n_sh = n // N_CORES
    with_bias = bool(np.any(bias != 0.0))
    nc = _get_nc(m_sh, k, n, with_bias)
    in_maps = [
        {
            "x": x[i * m_sh:(i + 1) * m_sh],
            "w": np.ascontiguousarray(weight[:, i * n_sh:(i + 1) * n_sh]),
            "b": bias,
        }
        for i in range(N_CORES)
    ]
    res = run_bass_kernel_spmd(nc, in_maps, core_ids=list(range(N_CORES)), trace=_trace, tmpdir=_tmpdir)
    out = np.concatenate([res.results[i]["y"] for i in range(N_CORES)], axis=0)
    if _trace:
        return out, res
    return out
